# revision 14
# baseline (speedup 1.0000x reference)
"""Trainium2 Bass kernel for nn_EpiNN_att (dense_transformer).

Math (per batch n, L=512, D=1280, D_hidden=32, 4 heads x head_dim 8):
    first_order[n]  = (x[n] @ w_token) . w_seq + b_seq
    h[n]            = x[n] @ W_proj.T                      # (L, 32)
    S[n]            = (h[n] @ h[n].T) * 1/(4*sqrt(8))      # mean-over-heads QK^T
    second_order[n] = interaction_scale * sum_{l<m} S[n,l,m] * esm[n,l,m]
    out[n]          = first_order[n] + second_order[n]

Key facts used:
  * einsum('nlhd,nmhd->nlm') over (head, head_dim) contracts ALL 32 channels,
    so the per-head-mean attention is just h @ h.T.
  * Sharding: data-parallel over N across 8 cores (8 batches each).
  * x is fed transposed (per-n (D, L)) so the D-contraction lands on SBUF
    partitions; an augmented weight [W_proj.T | w_token] yields (h^T,
    first-order row) in a single PE pass. The attention scale*interaction
    scale is folded into the column-sum selector matrix.
  * The strict-upper-triangular masked sum only needs the upper-block-column
    slices of S and esm (esm DMA skips 37.5%% of bytes); the per-n reduction
    runs on the PE as column-sum matmuls into one accumulator PSUM bank.
"""

import math

import numpy as np

N, L, D = 64, 512, 1280
DH = 32
N_HEADS, HEAD_DIM = 4, 8
SCALE = 1.0 / (N_HEADS * math.sqrt(HEAD_DIM))
NCORES = 8
NB = N // NCORES  # batches per core
KD = D // 128  # 10 contraction chunks
RL = L // 128  # 4 row chunks

# Compute dtype for the streamed tensors (x, esm, weights, h, T).
# "f16" halves DMA traffic and runs the PE at full rate; PSUM accumulation
# stays fp32 throughout. "f32" is the exact-reference fallback.
PRECISION = "f16"

_NC_CACHE = {}


def _np_dt(prec):
    return np.float16 if prec == "f16" else np.float32


def _build(prec):
    if prec in _NC_CACHE:
        return _NC_CACHE[prec]

    import concourse.mybir as mybir
    import concourse.tile as tile
    from concourse import bacc

    f32 = mybir.dt.float32
    dtc = mybir.dt.float16 if prec == "f16" else f32

    nc = bacc.Bacc()

    xT_d = nc.dram_tensor("xT", [NB, D, L], dtc, kind="ExternalInput")
    esm_d = nc.dram_tensor("esm", [NB, L, L], dtc, kind="ExternalInput")
    wT_d = nc.dram_tensor("wT", [D, 33], dtc, kind="ExternalInput")
    mm_d = nc.dram_tensor("mm", [33, L], f32, kind="ExternalInput")
    tri_d = nc.dram_tensor("tri", [128, 128], dtc, kind="ExternalInput")
    sel_d = nc.dram_tensor("sel", [128, NB * NB], dtc, kind="ExternalInput")
    selp_d = nc.dram_tensor("selp", [33, NB * NB], dtc, kind="ExternalInput")
    so_d = nc.dram_tensor("so_out", [NB, 1], f32, kind="ExternalOutput")

    with tile.TileContext(nc) as tc:
        with (
            tc.tile_pool(name="consts", bufs=1) as consts,
            tc.tile_pool(name="xpool", bufs=4) as xpool,
            tc.tile_pool(name="epool", bufs=8) as epool,
            tc.tile_pool(name="hwpool", bufs=3) as hwpool,
            tc.tile_pool(name="tpool", bufs=4) as tpool,
            tc.tile_pool(name="respool", bufs=1) as respool,
            tc.tile_pool(name="gpsum", bufs=2, space="PSUM") as gpsum,
            tc.tile_pool(name="spsum", bufs=3, space="PSUM") as spsum,
            tc.tile_pool(name="apsum", bufs=1, space="PSUM") as apsum,
        ):
            wT_sb = consts.tile([128, KD, 33], dtc)
            nc.sync.dma_start(out=wT_sb, in_=wT_d[:, :].rearrange("(k p) c -> p k c", p=128))
            mm_sb = consts.tile([33, L], f32)
            nc.sync.dma_start(out=mm_sb, in_=mm_d[:, :])
            tri_sb = consts.tile([128, 128], dtc)
            nc.sync.dma_start(out=tri_sb, in_=tri_d[:, :])
            sel_sb = consts.tile([128, NB * NB], dtc)
            nc.sync.dma_start(out=sel_sb, in_=sel_d[:, :])
            selp_sb = consts.tile([33, NB * NB], dtc)
            nc.sync.dma_start(out=selp_sb, in_=selp_d[:, :])

            acc = apsum.tile([NB, L], f32)

            for n in range(NB):
                xt = xpool.tile([128, KD, L], dtc)
                nc.sync.dma_start(out=xt, in_=xT_d[n, :, :].rearrange("(k p) l -> p k l", p=128))

                g = gpsum.tile([33, L], f32)
                for k in range(KD):
                    nc.tensor.matmul(
                        g, lhsT=wT_sb[:, k, :], rhs=xt[:, k, :],
                        start=(k == 0), stop=(k == KD - 1),
                    )

                # rows 0-31: h^T   row 32: fo1*w_seq
                hw = hwpool.tile([33, L], dtc)
                nc.vector.tensor_mul(hw, g, mm_sb)

                for r in range(RL):
                    rs = 128 * r
                    ncols = L - rs
                    et = epool.tile([128, L], dtc, tag="esm")
                    nc.scalar.dma_start(out=et[:, :ncols], in_=esm_d[n, rs : rs + 128, rs:L])

                    s = spsum.tile([128, L], f32)
                    nc.tensor.matmul(
                        s[:, :ncols],
                        lhsT=hw[0:32, rs : rs + 128],
                        rhs=hw[0:32, rs:L],
                        start=True, stop=True,
                    )

                    t = tpool.tile([128, L], dtc, tag="t")
                    nc.vector.tensor_mul(t[:, :ncols], s[:, :ncols], et[:, :ncols])
                    # strict-upper mask for the diagonal 128x128 block
                    nc.vector.tensor_mul(t[:, :128], t[:, :128], tri_sb)

                    nc.tensor.matmul(
                        acc[:, rs:L],
                        lhsT=sel_sb[:, NB * n : NB * (n + 1)],
                        rhs=t[:, :ncols],
                        start=(n == 0 and r == 0),
                        stop=(n == NB - 1 and r == RL - 1),
                    )

                nc.vector.reduce_sum(
                    out=fo_sb[32:33, n : n + 1],
                    in_=hw[32:33, :],
                    axis=mybir.AxisListType.X,
                )

            res = respool.tile([NB, 1], f32)
            nc.vector.reduce_sum(out=res, in_=acc, axis=mybir.AxisListType.X)
            nc.sync.dma_start(out=so_d[:, :], in_=res)
            nc.sync.dma_start(out=fo_d[:, :], in_=fo_sb[32:33, :])

    nc.compile()
    _NC_CACHE[prec] = nc
    return nc


def _prepare(x, esm_priors, w_token, w_seq, b_seq, W_proj, interaction_scale, prec):
    ndt = _np_dt(prec)
    alpha = SCALE * float(np.asarray(interaction_scale))

    # (N, D, L) so the contraction dim is partition-major on SBUF
    xT = np.ascontiguousarray(np.asarray(x).transpose(0, 2, 1)).astype(ndt)
    esm = np.ascontiguousarray(np.asarray(esm_priors)).astype(ndt)

    W = np.asarray(W_proj, np.float32)
    wT = np.concatenate(
        [W.T, np.asarray(w_token, np.float32)[:, None]], axis=1
    ).astype(ndt)  # (D, 33)
    mm = np.concatenate(
        [np.ones((32, L), np.float32), np.asarray(w_seq, np.float32)[None, :]], axis=0
    )  # (33, L)
    tri = np.triu(np.ones((128, 128), np.float32), k=1).astype(ndt)
    # alpha (attention scale * interaction_scale) rides on the selector so
    # the S matmul operands stay identical (same base partition)
    sel = np.zeros((128, NB * NB), np.float32)
    for n in range(NB):
        sel[:, NB * n + n] = alpha
    sel = sel.astype(ndt)
    selp = np.zeros((33, NB * NB), np.float32)
    for n in range(NB):
        selp[:, NB * n + n] = 1.0
    selp = selp.astype(ndt)

    in_maps = []
    for c in range(NCORES):
        in_maps.append(
            {
                "xT": xT[c * NB : (c + 1) * NB],
                "esm": esm[c * NB : (c + 1) * NB],
                "wT": wT,
                "mm": mm,
                "tri": tri,
                "sel": sel,
                "selp": selp,
            }
        )
    return in_maps


def _gather(results, b_seq):
    outs = [r["so_out"].ravel() for r in results]
    return (np.concatenate(outs) + np.float32(np.asarray(b_seq))).astype(np.float32)


def _run(trace=False, prec=None, reps=1, **inputs):
    from concourse.bass_utils import run_bass_kernel_spmd

    prec = prec or PRECISION
    nc = _build(prec, reps=reps)
    in_maps = _prepare(**inputs, prec=prec)
    res = run_bass_kernel_spmd(nc, in_maps, core_ids=list(range(NCORES)), trace=trace)
    out = _gather(res.results, inputs["b_seq"])
    return out, res


def kernel(**inputs) -> np.ndarray:
    out, _ = _run(trace=False, **inputs)
    return out
